# revision 5
# baseline (speedup 1.0000x reference)
"""Trainium2 kernel for nn_BatchedDTW — mixed bf16/fp8 stream.

kernel4 is DVE-bound (fp8 sub runs 1x). Streaming a tuned fraction of the
columns as bf16 lets DVE sub them at 2x, trading HBM bytes for DVE cycles:
cols [0, F16) arrive bf16, cols [F16, F) arrive fp8-e4m3.  With F16=1280
(31%), SD=256 squares on DVE, rest on ACT, all three of DMA (~3.9us),
DVE (~3.9us) and ACT (~3.9us) balance below kernel4's ~4.3us DVE bound.
sqrt stays software-pipelined one rep behind its square (kernel4's fix).
"""

from contextlib import ExitStack

import numpy as np
import ml_dtypes

import concourse.bass as bass
import concourse.mybir as mybir
from concourse.bass_utils import run_bass_kernel_spmd

N_CORES = 8
P = 128
C = 32
B, T, N = 4, 512, 64
ROWS = B * T * N // N_CORES   # 16384 rows per core
F = ROWS // 4                 # 4096 free cols per partition
F16 = 1280                    # cols streamed as bf16 (rest fp8)
F8 = F - F16
SL = 256                      # moving cols per matmul
NSL = F // SL                 # 16
SD = 256                      # cols squared on DVE (slice 0); rest ACT
KSETS = 4

_nc_cache = {}
_last_results = None


def _build(repeat=1, nbuf=None):
    if nbuf is None:
        nbuf = 2 if repeat > 1 else 1
    nc = bass.Bass()
    bf16 = mybir.dt.bfloat16
    f32 = mybir.dt.float32
    f8 = mybir.dt.float8e4
    z16_ext = nc.declare_dram_parameter("z16", [P, 2 * F16], bf16, isOutput=False)
    z8_ext = nc.declare_dram_parameter("z8", [P, 2 * F8], f8, isOutput=False)
    w_ext = nc.declare_dram_parameter("w", [P, 8 * 64], bf16, isOutput=False)
    out_ext = nc.declare_dram_parameter("out", [P, 1], f32, isOutput=True)

    ksets = min(repeat, KSETS)
    with ExitStack() as ctx:
        zt16 = ctx.enter_context(nc.sbuf_tensor([P, nbuf * 2 * F16], bf16))
        zt8 = ctx.enter_context(nc.sbuf_tensor([P, nbuf * 2 * F8], f8))
        df = ctx.enter_context(nc.sbuf_tensor([P, nbuf * F], bf16))
        sq = ctx.enter_context(nc.sbuf_tensor([P, nbuf * F], bf16))
        wt = ctx.enter_context(nc.sbuf_tensor([P, 8 * 64], bf16))
        acc = ctx.enter_context(nc.sbuf_tensor([P, nbuf], f32))
        ps = ctx.enter_context(nc.psum_tensor([P, nbuf * 2 * SL], f32))
        zsems = [ctx.enter_context(nc.semaphore(f"zsem{r}_{i}"))
                 for r in range(ksets) for i in range(2)]
        wsem = ctx.enter_context(nc.semaphore("wsem"))
        vsem = ctx.enter_context(nc.semaphore("vsem"))
        asem = ctx.enter_context(nc.semaphore("asem"))
        psem = ctx.enter_context(nc.semaphore("psem"))
        osem = ctx.enter_context(nc.semaphore("osem"))
        block = ctx.enter_context(nc.Block())

        def zs(r, ch):
            return zsems[(r % ksets) * 2 + ch]

        def z_done(r):
            return 16 * (r // ksets + 1)

        def o16(r):
            return (r % nbuf) * 2 * F16

        def o8(r):
            return (r % nbuf) * 2 * F8

        def foff(r):
            return (r % nbuf) * F

        def poff(r):
            return (r % nbuf) * 2 * SL

        # vsem: per rep sub16, sub8, mul
        def v_sub16_done(r):
            return 3 * r + 1

        def v_sub8_done(r):
            return 3 * r + 2

        def v_mul_done(r):
            return 3 * r + 3

        # asem (ACT pipelined): sq0, [sq1, sqrt0], [sq2, sqrt1], ..., sqrt(R-1)
        def a_sq_done(r):
            return 1 if r == 0 else 2 * r

        def a_sqrt_done(r):
            return 2 * repeat if r == repeat - 1 else 2 * r + 3

        def p_done(r, s):
            return NSL * r + s + 1

        @block.sync
        def _(sync):
            sync.dma_start(out=wt[:], in_=w_ext[:]).then_inc(wsem, 16)
            for r in range(repeat):
                if r >= nbuf:
                    sync.wait_ge(vsem, v_sub16_done(r - nbuf))
                sync.dma_start(
                    out=zt16[:, o16(r):o16(r) + 2 * F16],
                    in_=z16_ext[:],
                ).then_inc(zs(r, 0), 16)
                if r >= nbuf:
                    sync.wait_ge(vsem, v_sub8_done(r - nbuf))
                sync.dma_start(
                    out=zt8[:, o8(r):o8(r) + 2 * F8],
                    in_=z8_ext[:],
                ).then_inc(zs(r, 1), 16)

        @block.vector
        def _(vector):
            for r in range(repeat):
                vector.wait_ge(zs(r, 0), z_done(r))
                if r >= nbuf:
                    # WAR: df slot last read by ACT's square of rep r-nbuf
                    vector.wait_ge(asem, a_sq_done(r - nbuf))
                vector.tensor_sub(
                    df[:, foff(r):foff(r) + F16],
                    zt16[:, o16(r):o16(r) + F16],
                    zt16[:, o16(r) + F16:o16(r) + 2 * F16],
                ).then_inc(vsem, 1)
                vector.wait_ge(zs(r, 1), z_done(r))
                vector.tensor_sub(
                    df[:, foff(r) + F16:foff(r) + F],
                    zt8[:, o8(r):o8(r) + F8],
                    zt8[:, o8(r) + F8:o8(r) + 2 * F8],
                ).then_inc(vsem, 1)
                if r >= nbuf:
                    # WAR: sq slice 0 last read by PE matmul 0 of rep r-nbuf
                    vector.wait_ge(psem, p_done(r - nbuf, 0))
                vector.tensor_mul(
                    sq[:, foff(r):foff(r) + SD],
                    df[:, foff(r):foff(r) + SD],
                    df[:, foff(r):foff(r) + SD],
                ).then_inc(vsem, 1)

        @block.scalar
        def _(scalar):
            def emit_square(r):
                scalar.wait_ge(vsem, v_sub8_done(r))
                if r >= nbuf:
                    # WAR: sq cols [SD, F) last read by PE of rep r-nbuf
                    scalar.wait_ge(psem, p_done(r - nbuf, NSL - 1))
                scalar.square(
                    out=sq[:, foff(r) + SD:foff(r) + F],
                    in_=df[:, foff(r) + SD:foff(r) + F],
                ).then_inc(asem, 1)

            def emit_sqrt(r):
                scalar.wait_ge(psem, p_done(r, NSL - 1))
                scalar.activation(
                    out=ps[:, poff(r) + SL:poff(r) + 2 * SL],
                    in_=ps[:, poff(r):poff(r) + SL],
                    func=mybir.ActivationFunctionType.Sqrt,
                    accum_out=acc[:, r % nbuf:r % nbuf + 1],
                ).then_inc(asem, 1)

            for r in range(repeat):
                emit_square(r)
                if r >= 1:
                    emit_sqrt(r - 1)
            emit_sqrt(repeat - 1)
            scalar.wait_ge(asem, a_sqrt_done(repeat - 1))
            scalar.dma_start(
                out=out_ext[:],
                in_=acc[:, (repeat - 1) % nbuf:(repeat - 1) % nbuf + 1],
            ).then_inc(osem, 16)
            scalar.wait_ge(osem, 16)

        @block.tensor
        def _(tensor):
            tensor.wait_ge(wsem, 16)
            for r in range(repeat):
                for s in range(NSL):
                    h, i = s // 8, s % 8
                    if s == 0:
                        # producer: DVE's mul covers sq slice 0
                        tensor.wait_ge(vsem, v_mul_done(r))
                    elif s == 1:
                        # producer: ACT's square covers [SD, F)
                        tensor.wait_ge(asem, a_sq_done(r))
                    if i == 0 and r >= nbuf:
                        # WAR: psum half reset; sqrt(r-nbuf) must have read it
                        tensor.wait_ge(asem, a_sqrt_done(r - nbuf))
                    tensor.matmul(
                        out=ps[64 * h:64 * (h + 1), poff(r):poff(r) + SL],
                        lhsT=wt[:, 64 * i:64 * (i + 1)],
                        rhs=sq[:, foff(r) + s * SL:foff(r) + (s + 1) * SL],
                        start=(i == 0),
                        stop=(i == 7),
                    ).then_inc(psem, 1)
    return nc


def make_weights():
    w = np.zeros((P, 8 * 64), dtype=np.float32)
    k = np.arange(P)
    for i in range(8):
        w[k, 64 * i + 4 * i + k // C] = 1.0
    return w.astype(ml_dtypes.bfloat16)


def pack_inputs(X, Y):
    def to_parts(A):
        A = np.asarray(A, dtype=np.float32).reshape(N_CORES, F, 4, C)
        return A.transpose(0, 2, 3, 1).reshape(N_CORES, P, F)

    Xp, Yp = to_parts(X), to_parts(Y)
    Z16 = np.concatenate([Xp[:, :, :F16], Yp[:, :, :F16]], axis=2)
    Z8 = np.concatenate([Xp[:, :, F16:], Yp[:, :, F16:]], axis=2)
    return (Z16.astype(ml_dtypes.bfloat16), Z8.astype(ml_dtypes.float8_e4m3))


def kernel(X, Y, window=None, **_):
    global _nc_cache, _last_results
    Z16, Z8 = pack_inputs(X, Y)
    W = make_weights()
    if "k" not in _nc_cache:
        _nc_cache["k"] = _build()
    in_maps = [{"z16": Z16[k], "z8": Z8[k], "w": W} for k in range(N_CORES)]
    res = run_bass_kernel_spmd(_nc_cache["k"], in_maps, list(range(N_CORES)))
    _last_results = res
    partials = np.stack([r["out"] for r in res.results])
    total = partials.astype(np.float64).sum()
    return np.float32(total / (B * N))
